# revision 1
# baseline (speedup 1.0000x reference)
"""Trainium2 Bass kernel for DecomposingAttnProcessor (pad variant).

Math (pad branch contributes exactly zero since pad tokens are zeros
projected with no bias -> k_pad = v_pad = 0):
    q = hs @ Wq.T / (temp + eps)   (scale folded into Wq on host)
    k = ehs @ Wk.T ; v = ehs @ Wv.T
    scores[c,h,s,e] = q . k        (per head, dh=64)
    w = softmax over the 4 components c (dim 0)
    o = w @ v ; out = o @ Wo.T + bo + hs

Sharding: 8 cores, split S=4096 into 512-row blocks; all 4 components of
a block stay on one core (softmax couples them). K/V computed redundantly
per core (encoder seq is only 154).

Device layout is fully transposed (features on partitions): inputs are
host-transposed, output is produced transposed and host-untransposed.
"""

import numpy as np
import ml_dtypes

import concourse.bass as bass
import concourse.mybir as mybir
import concourse.tile as tile
from concourse import bacc
from concourse.bass_utils import run_bass_kernel_spmd

F32 = mybir.dt.float32
F32R = mybir.dt.float32r
BF16 = mybir.dt.bfloat16
AF = mybir.ActivationFunctionType
ALU = mybir.AluOpType

NCOMP = 4
HEADS = 24
DH = 64
D = 1536
S = 4096
E = 154
EPS = 1e-8
NCORES = 8
SL = S // NCORES          # 512 s-rows per core (per component)
SH = SL // 2              # 256: s-half tile width (N of most matmuls)
FT = D // 128             # 12 feature tiles of 128
HP = HEADS // 2           # 12 head-pairs (2 heads = 128 feature rows)
ECAT = NCOMP * E          # 616: components stacked along encoder axis
ETILES = ((0, 128), (128, E - 128))   # e split: 128 + 26


def _emit(tc):
    import os
    phases = os.environ.get("K_PHASES", "ABC")
    blevel = int(os.environ.get("K_BLEVEL", "4"))
    nc = tc.nc

    xT = nc.declare_dram_parameter("xT", [NCOMP, D, SL], F32, isOutput=False)
    xTb = nc.declare_dram_parameter("xTb", [NCOMP, D, SL], BF16, isOutput=False)
    eT = nc.declare_dram_parameter("eT", [D, ECAT], BF16, isOutput=False)
    wqT = nc.declare_dram_parameter("wqT", [D, D], BF16, isOutput=False)
    wkT = nc.declare_dram_parameter("wkT", [D, D], BF16, isOutput=False)
    wvT = nc.declare_dram_parameter("wvT", [D, D], BF16, isOutput=False)
    woT = nc.declare_dram_parameter("woT", [D, D], BF16, isOutput=False)
    bo = nc.declare_dram_parameter("bo", [128, FT], F32, isOutput=False)
    outT = nc.declare_dram_parameter("outT", [NCOMP, D, SL], F32, isOutput=True)

    # DRAM views with the 128-row tile index folded into the free dim, so a
    # whole [1536, n] panel loads as one DMA into a [128, FT*n] tile.
    xT_v = [xT[c].rearrange("(f p) s -> p f s", p=128) for c in range(NCOMP)]
    xTb_v = [xTb[c].rearrange("(f p) s -> p f s", p=128) for c in range(NCOMP)]
    eT_v = eT.rearrange("(f p) e -> p f e", p=128)
    wqT_v = wqT.rearrange("(f p) o -> p f o", p=128)
    wkT_v = wkT.rearrange("(f p) o -> p f o", p=128)
    wvT_v = wvT.rearrange("(f p) o -> p f o", p=128)
    woT_v = woT.rearrange("(f p) o -> p f o", p=128)
    outT_v = [outT[c].rearrange("(f p) s -> p f s", p=128) for c in range(NCOMP)]

    with tc.tile_pool(name="persist", bufs=1) as pp:
        # ---------------- persistent tiles ----------------
        kt_sb = [pp.tile([128, ECAT], BF16, tag="kT", bufs=FT, name=f"kt{t}")
                 for t in range(FT)]
        v_sb = [[pp.tile([esz, D], BF16, tag=f"v{ei}", bufs=NCOMP,
                         name=f"v{c}_{ei}")
                 for ei, (eo, esz) in enumerate(ETILES)] for c in range(NCOMP)]
        bo_sb = pp.tile([128, FT], F32, tag="bo", bufs=1, name="bo_sb")
        nc.sync.dma_start(out=bo_sb[:], in_=bo[:])

        def _phases():
            # ---------------- phase A: K^T and V ----------------
            if "A" in phases:
              with (
                tc.tile_pool(name="pha", bufs=1) as pa,
                tc.tile_pool(name="pha_psum", bufs=1, space="PSUM") as pap,
              ):
                et_b = pa.tile([128, FT * ECAT], BF16, tag="eT", bufs=1,
                               name="et_b")
                nc.sync.dma_start(
                    out=et_b.rearrange("p (f e) -> p f e", f=FT), in_=eT_v)
                et = [et_b[:, fi * ECAT:(fi + 1) * ECAT] for fi in range(FT)]

                # K^T[fo, c*E + e] over fi; N split 308+308 (>=256 keeps f32r
                # at full rate)
                for fot in range(FT):
                    wk_b = pa.tile([128, FT * 128], BF16, tag="wk", bufs=3,
                                   name=f"wk{fot}")
                    nc.sync.dma_start(
                        out=wk_b.rearrange("p (f o) -> p f o", f=FT),
                        in_=wkT_v[:, :, fot * 128:(fot + 1) * 128])
                    for nch in range(2):
                        n0 = nch * 308
                        pk = pap.tile([128, 308], F32, tag="pk", bufs=2,
                                      name=f"pk{fot}_{nch}")
                        for fi in range(FT):
                            nc.tensor.matmul(
                                pk[:], wk_b[:, fi * 128:(fi + 1) * 128],
                                et[fi][:, n0:n0 + 308],
                                start=(fi == 0), stop=(fi == FT - 1))
                        nc.vector.tensor_copy(
                            out=kt_sb[fot][:, n0:n0 + 308], in_=pk[:])

                # V[c][e, fv] (natural layout, bf16) over fi
                for fvc in range(3):
                    wv_b = pa.tile([128, FT * 512], BF16, tag="wv", bufs=2,
                                   name=f"wv{fvc}")
                    nc.sync.dma_start(
                        out=wv_b.rearrange("p (f o) -> p f o", f=FT),
                        in_=wvT_v[:, :, fvc * 512:(fvc + 1) * 512])
                    for c in range(NCOMP):
                        for ei, (eo, esz) in enumerate(ETILES):
                            pv = pap.tile([128, 512], F32, tag="pv", bufs=2,
                                          name=f"pv{fvc}_{c}_{ei}")
                            for fi in range(FT):
                                nc.tensor.matmul(
                                    pv[:esz, :],
                                    et[fi][:, c * E + eo:c * E + eo + esz],
                                    wv_b[:, fi * 512:(fi + 1) * 512],
                                    start=(fi == 0), stop=(fi == FT - 1))
                            nc.vector.tensor_copy(
                                out=v_sb[c][ei][:, fvc * 512:(fvc + 1) * 512],
                                in_=pv[:esz, :])

            # ---------------- phases B+C per s-half ----------------
            with (
                tc.tile_pool(name="bc", bufs=1) as bc,
                tc.tile_pool(name="bcp", bufs=1, space="PSUM") as bcp,
            ):
                for half in range(2):
                    s0 = half * SH
                    # bf16 x^T panels for the Q projection
                    xh = []
                    for c in range(NCOMP):
                        t = bc.tile([128, FT * SH], BF16, tag="xh", bufs=5,
                                    name=f"xh{half}_{c}")
                        nc.sync.dma_start(
                            out=t.rearrange("p (f s) -> p f s", f=FT),
                            in_=xTb_v[c][:, :, s0:s0 + SH])
                        xh.append(t)

                    # -------- phase B: Q, scores, softmax, o --------
                    ot_sb = {}
                    for hp in range(HP if "B" in phases else 0):
                        wq_b = bc.tile([128, FT * 128], BF16, tag="wq", bufs=2,
                                       name=f"wq{half}_{hp}")
                        nc.sync.dma_start(
                            out=wq_b.rearrange("p (f o) -> p f o", f=FT),
                            in_=wqT_v[:, :, hp * 128:(hp + 1) * 128])

                        # Q^T for the two heads of this pair, all 4 components
                        qt = []
                        for c in range(NCOMP):
                            pq = bcp.tile([128, SH], F32, tag="pq", bufs=2,
                                          name=f"pq{half}_{hp}_{c}")
                            for fi in range(FT):
                                nc.tensor.matmul(
                                    pq[:], wq_b[:, fi * 128:(fi + 1) * 128],
                                    xh[c][:, fi * SH:(fi + 1) * SH],
                                    start=(fi == 0), stop=(fi == FT - 1))
                            q = bc.tile([128, SH], BF16, tag="qT", bufs=6,
                                        name=f"qt{half}_{hp}_{c}")
                            nc.scalar.copy(q[:], pq[:])
                            qt.append(q)
                        if blevel < 2:
                            continue

                        # scores + exp, both heads packed along the free dim
                        exps = [[None, None] for _ in range(NCOMP)]
                        for ei, (eo, esz) in enumerate(ETILES):
                            for c in range(NCOMP):
                                # separate psum banks per head: matmul psum
                                # writes must start at a bank boundary
                                ex = bc.tile([esz, 2 * SH], BF16, tag=f"exp{ei}",
                                             bufs=6, name=f"ex{half}_{hp}_{ei}_{c}")
                                for hh in range(2):
                                    ps = bcp.tile([128, SH], F32, tag="ps",
                                                  bufs=3,
                                                  name=f"ps{half}_{hp}_{ei}_{c}_{hh}")
                                    nc.tensor.matmul(
                                        ps[:esz, :],
                                        kt_sb[hp][hh * 64:(hh + 1) * 64,
                                                  c * E + eo:c * E + eo + esz],
                                        qt[c][hh * 64:(hh + 1) * 64, :],
                                        start=True, stop=True)
                                    nc.scalar.activation(
                                        ex[:, hh * SH:(hh + 1) * SH],
                                        ps[:esz, :], AF.Exp)
                                exps[c][ei] = ex
                            ssum = bc.tile([esz, 2 * SH], BF16, tag=f"sum{ei}",
                                           bufs=4, name=f"sm{half}_{hp}_{ei}")
                            nc.vector.tensor_add(out=ssum[:], in0=exps[0][ei][:],
                                                 in1=exps[1][ei][:])
                            nc.vector.tensor_add(out=ssum[:], in0=ssum[:],
                                                 in1=exps[2][ei][:])
                            nc.vector.tensor_add(out=ssum[:], in0=ssum[:],
                                                 in1=exps[3][ei][:])
                            rinv = bc.tile([esz, 2 * SH], BF16, tag=f"sum{ei}",
                                           bufs=4, name=f"ri{half}_{hp}_{ei}")
                            with nc.allow_low_precision(
                                    reason="softmax weights are consumed in bf16"):
                                nc.vector.reciprocal(out=rinv[:], in_=ssum[:])
                            for c in range(NCOMP):
                                w = bc.tile([esz, 2 * SH], BF16, tag=f"w{ei}",
                                            bufs=6, name=f"w{half}_{hp}_{ei}_{c}")
                                nc.vector.tensor_mul(out=w[:], in0=exps[c][ei][:],
                                                     in1=rinv[:])
                                exps[c][ei] = w  # normalized weights

                        # o^T: V-slices @ w; head hh lands on psum partitions
                        # hh*64..hh*64+64 (own accumulation group per head, both
                        # column-aligned to the bank start)
                        for c in range(NCOMP if blevel >= 4 else 0):
                            po = bcp.tile([128, SH], F32, tag="po", bufs=2,
                                          name=f"po{half}_{hp}_{c}")
                            for hh in range(2):
                                h = hp * 2 + hh
                                for ei, (eo, esz) in enumerate(ETILES):
                                    nc.tensor.matmul(
                                        po[hh * 64:(hh + 1) * 64, :],
                                        v_sb[c][ei][:, h * 64:(h + 1) * 64],
                                        exps[c][ei][:, hh * SH:(hh + 1) * SH],
                                        start=(ei == 0), stop=(ei == 1),
                                        skip_group_check=True)
                            ot = bc.tile([128, SH], BF16, tag="oT", bufs=48,
                                         name=f"ot{half}_{hp}_{c}")
                            nc.vector.tensor_copy(out=ot[:], in_=po[:])
                            ot_sb[(c, hp)] = ot

                    # -------- phase C: out-proj + bias + residual --------
                    for fot in range(FT if "C" in phases else 0):
                        wo_b = bc.tile([128, FT * 128], BF16, tag="wo", bufs=3,
                                       name=f"wo{half}_{fot}")
                        nc.sync.dma_start(
                            out=wo_b.rearrange("p (f o) -> p f o", f=FT),
                            in_=woT_v[:, :, fot * 128:(fot + 1) * 128])
                        for c in range(NCOMP):
                            xr = bc.tile([128, SH], F32, tag="xr", bufs=4,
                                         name=f"xr{half}_{fot}_{c}")
                            nc.sync.dma_start(
                                out=xr[:],
                                in_=xT_v[c][:, fot, s0:s0 + SH])
                            po = bcp.tile([128, SH], F32, tag="pout", bufs=1,
                                          name=f"pc{half}_{fot}_{c}")
                            for fi in range(FT):
                                nc.tensor.matmul(
                                    po[:], wo_b[:, fi * 128:(fi + 1) * 128],
                                    ot_sb[(c, fi)][:],
                                    start=(fi == 0), stop=(fi == FT - 1))
                            ob = bc.tile([128, SH], F32, tag="outsb", bufs=4,
                                         name=f"ob{half}_{fot}_{c}")
                            nc.vector.scalar_tensor_tensor(
                                out=ob[:], in0=po[:],
                                scalar=bo_sb[:, fot:fot + 1],
                                in1=xr[:],
                                op0=ALU.add, op1=ALU.add)
                            nc.sync.dma_start(
                                out=outT_v[c][:, fot, s0:s0 + SH], in_=ob[:])


        repeat = int(os.environ.get("K_REPEAT", "1"))
        for _rep in range(repeat):
            _phases()


_NC_CACHE = {}


def _get_nc():
    if "nc" not in _NC_CACHE:
        nc = bacc.Bacc("TRN2", target_bir_lowering=False)
        with tile.TileContext(nc) as tc:
            _emit(tc)
        nc.compile()
        _NC_CACHE["nc"] = nc
    return _NC_CACHE["nc"]


def kernel(hidden_states, encoder_hidden_states, temperature, Wq, Wk, Wv, Wo,
           bo, pad_length):
    # pad branch contributes zero to the output (zeros projected with no
    # bias give k_pad = v_pad = 0), so pad_length is irrelevant.
    hs = np.ascontiguousarray(np.asarray(hidden_states, dtype=np.float32))
    ehs = np.ascontiguousarray(
        np.asarray(encoder_hidden_states, dtype=np.float32))
    temp = float(np.asarray(temperature).reshape(-1)[0])
    Wq = np.asarray(Wq, dtype=np.float32)
    Wk = np.asarray(Wk, dtype=np.float32)
    Wv = np.asarray(Wv, dtype=np.float32)
    Wo = np.asarray(Wo, dtype=np.float32)
    bo_v = np.asarray(bo, dtype=np.float32).reshape(-1)

    wqT = np.ascontiguousarray((Wq / (temp + EPS)).T).astype(ml_dtypes.bfloat16)
    wkT = np.ascontiguousarray(Wk.T).astype(ml_dtypes.bfloat16)
    wvT = np.ascontiguousarray(Wv.T).astype(ml_dtypes.bfloat16)
    woT = np.ascontiguousarray(Wo.T).astype(ml_dtypes.bfloat16)
    eT_all = np.ascontiguousarray(
        np.concatenate([ehs[c].T for c in range(NCOMP)],
                       axis=1)).astype(ml_dtypes.bfloat16)
    bo_t = np.ascontiguousarray(bo_v.reshape(FT, 128).T)

    nc = _get_nc()
    in_maps = []
    for i in range(NCORES):
        xT_i = np.ascontiguousarray(
            hs[:, i * SL:(i + 1) * SL, :].transpose(0, 2, 1))
        in_maps.append({
            "xT": xT_i, "xTb": xT_i.astype(ml_dtypes.bfloat16),
            "eT": eT_all, "wqT": wqT, "wkT": wkT,
            "wvT": wvT, "woT": woT, "bo": bo_t,
        })

    res = run_bass_kernel_spmd(nc, in_maps, core_ids=list(range(NCORES)))

    out = np.empty((NCOMP, S, D), dtype=np.float32)
    for i in range(NCORES):
        out[:, i * SL:(i + 1) * SL, :] = res.results[i]["outT"].transpose(
            0, 2, 1)
    return out



# revision 10
# speedup vs baseline: 1.6036x; 1.6036x over previous
"""Trainium2 Bass kernel for DecomposingAttnProcessor (pad variant).

Math (pad branch contributes exactly zero since pad tokens are zeros
projected with no bias -> k_pad = v_pad = 0):
    q = hs @ Wq.T / (temp + eps)   (scale folded into Wq on host)
    k = ehs @ Wk.T ; v = ehs @ Wv.T
    scores[c,h,s,e] = q . k        (per head, dh=64)
    w = softmax over the 4 components c (dim 0)
    o = w @ v ; out = o @ Wo.T + bo + hs

Sharding: 8 cores, split S=4096 into 512-row blocks; all 4 components of
a block stay on one core (softmax couples them). K/V computed redundantly
per core (encoder seq is only 154).

Device layout is fully transposed (features on partitions). The encoder
axis is zero-padded per component to EP=160 on host so the e-tail
(154-128=26 rows) becomes a clean 32-wide strip: tail scores / V / o
matmuls pack the 4 components into the four 32-col (or 32-row) PE array
tile groups and run concurrently; the pad rows contribute exactly zero
(k_pad = v_pad = 0).
"""

import numpy as np
import ml_dtypes

import concourse.bass as bass
import concourse.mybir as mybir
import concourse.tile as tile
from concourse import bacc
from concourse.bass_utils import run_bass_kernel_spmd

F32 = mybir.dt.float32
BF16 = mybir.dt.bfloat16
AF = mybir.ActivationFunctionType
ALU = mybir.AluOpType

NCOMP = 4
HEADS = 24
DH = 64
D = 1536
S = 4096
E = 154
E0 = 128                  # head chunk of encoder axis
EP = 160                  # per-component padded encoder length
E1 = EP - E0              # 32: tail chunk (rows 154..159 are zero pad)
EPCAT = NCOMP * EP        # 640
EH = EPCAT // 2           # 320: K^T psum chunk
EPS = 1e-8
NCORES = 8
SL = S // NCORES          # 512 s-rows per core (per component)
FT = D // 128             # 12 feature tiles of 128
HP = HEADS // 2           # 12 head-pairs (2 heads = 128 feature rows)


def _emit(tc):
    nc = tc.nc

    xT = nc.declare_dram_parameter("xT", [NCOMP, D, SL], F32, isOutput=False)
    xTb = nc.declare_dram_parameter("xTb", [NCOMP, D, SL], BF16, isOutput=False)
    eT = nc.declare_dram_parameter("eT", [D, EPCAT], BF16, isOutput=False)
    wqT = nc.declare_dram_parameter("wqT", [D, D], BF16, isOutput=False)
    wkT = nc.declare_dram_parameter("wkT", [D, D], BF16, isOutput=False)
    wvT = nc.declare_dram_parameter("wvT", [D, D], BF16, isOutput=False)
    woT = nc.declare_dram_parameter("woT", [D, D], BF16, isOutput=False)
    bo = nc.declare_dram_parameter("bo", [128, FT], F32, isOutput=False)
    outT = nc.declare_dram_parameter("outT", [NCOMP, D, SL], F32, isOutput=True)

    # DRAM views with the 128-row tile index folded into the free dim.
    xT_v = xT.rearrange("c (f p) s -> p c f s", p=128)
    xTb_v = [xTb[c].rearrange("(f p) s -> p f s", p=128) for c in range(NCOMP)]
    eT_v = eT.rearrange("(f p) e -> p f e", p=128)
    wqT_v = wqT.rearrange("(f p) o -> p f o", p=128)
    wkT_v = wkT.rearrange("(f p) o -> p f o", p=128)
    wvT_v = wvT.rearrange("(f p) o -> p f o", p=128)
    woT_v = woT.rearrange("(f p) o -> p f o", p=128)
    outT_v = outT.rearrange("c (f p) s -> p c f s", p=128)

    with (
        tc.tile_pool(name="persist", bufs=1) as pp,
        tc.tile_pool(name="psum", bufs=1, space="PSUM") as qp,
    ):
        # ---------------- persistent tiles ----------------
        # K^T per head-pair: partitions = the 128 K-features of the pair,
        # free = c*EP + e.
        kt_sb = [pp.tile([128, EPCAT], BF16, tag="kT", bufs=FT, name=f"kt{t}")
                 for t in range(FT)]
        # V head chunk: partitions = e rows 0..127 of comp c, free = all
        # 1536 v-features.
        v0_sb = [pp.tile([128, D], BF16, tag="v0", bufs=NCOMP, name=f"v0_{c}")
                 for c in range(NCOMP)]
        # V tail: partitions 32c..32c+31 = e rows 128..159 of comp c
        # (rows >=154 are zeros).
        v1_sb = pp.tile([128, D], BF16, tag="v1", bufs=1, name="v1_sb")
        bo_sb = pp.tile([128, FT], F32, tag="bo", bufs=1, name="bo_sb")
        nc.sync.dma_start(out=bo_sb[:], in_=bo[:])

        def _copy3(i, out, in_):
            # alternate PSUM->SBUF copies over vector/scalar (gpsimd cannot
            # read PSUM)
            if i % 2 == 0:
                nc.vector.tensor_copy(out=out, in_=in_)
            else:
                nc.scalar.copy(out, in_)


        # ---------------- phase A: K^T and V ----------------
        with tc.tile_pool(name="pha", bufs=1) as pa:
            et_b = pa.tile([128, FT * EPCAT], BF16, tag="eT", bufs=1,
                           name="et_b")
            nc.sync.dma_start(
                out=et_b.rearrange("p (f e) -> p f e", f=FT), in_=eT_v)
            et = [et_b[:, fi * EPCAT:(fi + 1) * EPCAT] for fi in range(FT)]

            # K^T[fo, c*EP + e] over fi; N=640 split 320+320 per psum bank.
            for fot in range(FT):
                wk_b = pa.tile([128, FT * 128], BF16, tag="wk", bufs=3,
                               name=f"wk{fot}")
                nc.sync.dma_start(
                    out=wk_b.rearrange("p (f o) -> p f o", f=FT),
                    in_=wkT_v[:, :, fot * 128:(fot + 1) * 128])
                pk = qp.tile([128, 1024], F32, tag="ps2", bufs=2,
                             name=f"pk{fot}")
                for nch in range(2):
                    for fi in range(FT):
                        nc.tensor.matmul(
                            pk[:, nch * 512:nch * 512 + EH],
                            wk_b[:, fi * 128:(fi + 1) * 128],
                            et[fi][:, nch * EH:(nch + 1) * EH],
                            start=(fi == 0), stop=(fi == FT - 1))
                _copy3(fot, kt_sb[fot].rearrange("p (h n) -> p h n", h=2),
                       pk.rearrange("p (h n) -> p h n", h=2)[:, :, 0:EH])

            # V over fi, 512 v-feature columns at a time.
            for fvc in range(3):
                wv_b = pa.tile([128, FT * 512], BF16, tag="wv", bufs=2,
                               name=f"wv{fvc}")
                nc.sync.dma_start(
                    out=wv_b.rearrange("p (f o) -> p f o", f=FT),
                    in_=wvT_v[:, :, fvc * 512:(fvc + 1) * 512])
                for c in range(NCOMP):
                    pv = qp.tile([128, 512], F32, tag="pq", bufs=2,
                                 name=f"pv{fvc}_{c}")
                    for fi in range(FT):
                        nc.tensor.matmul(
                            pv[:], et[fi][:, c * EP:c * EP + E0],
                            wv_b[:, fi * 512:(fi + 1) * 512],
                            start=(fi == 0), stop=(fi == FT - 1))
                    _copy3(c, v0_sb[c][:, fvc * 512:(fvc + 1) * 512], pv[:])
                # tail: 4 components col-packed into one psum bank
                pv1 = qp.tile([128, 512], F32, tag="po", bufs=2,
                              name=f"pv1_{fvc}")
                for fi in range(FT):
                    for c in range(NCOMP):
                        nc.tensor.matmul(
                            pv1[32 * c:32 * c + E1, :],
                            et[fi][:, c * EP + E0:(c + 1) * EP],
                            wv_b[:, fi * 512:(fi + 1) * 512],
                            start=(fi == 0), stop=(fi == FT - 1),
                            skip_group_check=True,
                            tile_position=(0, 32 * c))
                _copy3(fvc, v1_sb[:, fvc * 512:(fvc + 1) * 512], pv1[:])

        # ---------------- phases B (attention) + C (out-proj) ----------------
        with tc.tile_pool(name="phb", bufs=1) as pb:
            ot_sb = {}

            def emit_o(hp):
                # o^T for head pair hp: psum partitions hh*64..hh*64+64 per
                # head (col groups), accumulation over the two e-chunks.
                # Emission order: all ei0 (full-K, col-paired), then all ei1
                # (K=32 row strips x col groups -> 8 concurrent matmuls).
                pos = []
                for c in range(NCOMP):
                    po = qp.tile([128, 512], F32, tag="po", bufs=2,
                                 name=f"po{hp}_{c}")
                    for hh in range(2):
                        h = hp * 2 + hh
                        nc.tensor.matmul(
                            po[hh * 64:(hh + 1) * 64, :],
                            v0_sb[c][:, h * 64:(h + 1) * 64],
                            ex0_sb[(hp, c)][:, hh * 512:(hh + 1) * 512],
                            start=True, stop=False, skip_group_check=True)
                    pos.append(po)
                for c in range(NCOMP):
                    for hh in range(2):
                        h = hp * 2 + hh
                        nc.tensor.matmul(
                            pos[c][hh * 64:(hh + 1) * 64, :],
                            v1_sb[32 * c:32 * c + E1, h * 64:(h + 1) * 64],
                            ex1_sb[hp][32 * c:32 * c + E1,
                                       hh * 512:(hh + 1) * 512],
                            start=False, stop=True, skip_group_check=True,
                            tile_position=(32 * c, 64 * hh))
                for c in range(NCOMP):
                    ot = pb.tile([128, 512], BF16, tag="oT", bufs=48,
                                 name=f"ot{hp}_{c}")
                    if c % 2 == 0:
                        nc.vector.tensor_copy(out=ot[:], in_=pos[c][:])
                    else:
                        nc.scalar.copy(ot[:], pos[c][:])
                    ot_sb[(c, hp)] = ot

            ex0_sb = {}
            ex1_sb = {}
            with tc.tile_pool(name="phbx", bufs=1) as pbx:
                xh = []
                for c in range(NCOMP):
                    t = pbx.tile([128, FT * SL], BF16, tag="xh", bufs=NCOMP,
                                 name=f"xh{c}")
                    nc.gpsimd.dma_start(
                        out=t.rearrange("p (f s) -> p f s", f=FT),
                        in_=xTb_v[c])
                    xh.append(t)

                for hp in range(HP):
                    wq_b = pb.tile([128, FT * 128], BF16, tag="wq", bufs=2,
                                   name=f"wq{hp}")
                    nc.sync.dma_start(
                        out=wq_b.rearrange("p (f o) -> p f o", f=FT),
                        in_=wqT_v[:, :, hp * 128:(hp + 1) * 128])

                    # Q^T for the two heads of this pair, all 4 components
                    qt = []
                    for c in range(NCOMP):
                        pq = qp.tile([128, 512], F32, tag="pq", bufs=2,
                                     name=f"pq{hp}_{c}")
                        for fi in range(FT):
                            nc.tensor.matmul(
                                pq[:], wq_b[:, fi * 128:(fi + 1) * 128],
                                xh[c][:, fi * SL:(fi + 1) * SL],
                                start=(fi == 0), stop=(fi == FT - 1))
                        q = pb.tile([128, SL], BF16, tag="qT", bufs=6,
                                    name=f"qt{hp}_{c}")
                        nc.scalar.copy(q[:], pq[:])
                        qt.append(q)

                    # scores head-chunk: per c one 2-bank psum, the two heads
                    # row-packed (K=64 at row offsets 0/64).
                    for c in range(NCOMP):
                        ps = qp.tile([128, 1024], F32, tag="ps2", bufs=2,
                                     name=f"ps{hp}_{c}")
                        for hh in range(2):
                            nc.tensor.matmul(
                                ps[:, hh * 512:(hh + 1) * 512],
                                kt_sb[hp][hh * 64:(hh + 1) * 64,
                                          c * EP:c * EP + E0],
                                qt[c][hh * 64:(hh + 1) * 64, :],
                                start=True, stop=True)
                        ex = pb.tile([128, 1024], BF16, tag="ex0", bufs=8,
                                     name=f"ex0_{hp}_{c}")
                        for hh in range(2):
                            nc.scalar.activation(
                                ex[:, hh * 512:(hh + 1) * 512],
                                ps[:, hh * 512:(hh + 1) * 512], AF.Exp)
                        ex0_sb[(hp, c)] = ex

                    # scores tail: 8 concurrent matmuls (rows 64*hh x col
                    # group c), all into one 2-bank psum.
                    ps1 = qp.tile([128, 1024], F32, tag="ps2", bufs=2,
                                  name=f"ps1_{hp}")
                    for c in range(NCOMP):
                        for hh in range(2):
                            nc.tensor.matmul(
                                ps1[32 * c:32 * c + E1,
                                    hh * 512:(hh + 1) * 512],
                                kt_sb[hp][hh * 64:(hh + 1) * 64,
                                          c * EP + E0:(c + 1) * EP],
                                qt[c][hh * 64:(hh + 1) * 64, :],
                                start=True, stop=True,
                                skip_group_check=True,
                                tile_position=(64 * hh, 32 * c))
                    ex1 = pb.tile([128, 1024], BF16, tag="ex1", bufs=3,
                                  name=f"ex1_{hp}")
                    for hh in range(2):
                        nc.scalar.activation(
                            ex1[:, hh * 512:(hh + 1) * 512],
                            ps1[:, hh * 512:(hh + 1) * 512], AF.Exp)
                    ex1_sb[hp] = ex1

                    # cross-component softmax: sum the 4 exps, fast
                    # reciprocal, scale each exp (weights stay in-place).
                    e0 = [ex0_sb[(hp, c)] for c in range(NCOMP)]
                    sa = pb.tile([128, 1024], F32, tag="sa", bufs=2,
                                 name=f"sa{hp}")
                    sb_ = pb.tile([128, 1024], F32, tag="sb", bufs=2,
                                  name=f"sb{hp}")
                    nc.vector.tensor_add(out=sa[:], in0=e0[0][:], in1=e0[1][:])
                    nc.gpsimd.tensor_add(out=sb_[:], in0=e0[2][:],
                                         in1=e0[3][:])
                    nc.vector.tensor_add(out=sa[:], in0=sa[:], in1=sb_[:])
                    rinv0 = pb.tile([128, 1024], F32, tag="sa", bufs=2,
                                    name=f"ri0_{hp}")
                    nc.vector.reciprocal_approx_fast(out=rinv0[:], in_=sa[:])
                    rb0 = pb.tile([128, 1024], BF16, tag="rb0", bufs=2,
                                  name=f"rb0_{hp}")
                    nc.scalar.copy(rb0[:], rinv0[:])

                    # tail: components live on 32-partition strips (e-row
                    # j of comp c at partition 32c+j). Element-wise engines
                    # need same-partition operands, so the cross-strip moves
                    # go through SBUF->SBUF DMA.
                    m1 = pb.tile([64, 1024], BF16, tag="m1", bufs=2,
                                 name=f"m1_{hp}")
                    nc.sync.dma_start(out=m1[:], in_=ex1[64:128, :])
                    t1 = pb.tile([64, 1024], BF16, tag="s1", bufs=2,
                                 name=f"t1_{hp}")
                    nc.vector.tensor_add(out=t1[:], in0=ex1[0:64, :],
                                         in1=m1[:])
                    m2 = pb.tile([32, 1024], BF16, tag="m2", bufs=2,
                                 name=f"m2_{hp}")
                    nc.sync.dma_start(out=m2[:], in_=t1[32:64, :])
                    s1 = pb.tile([32, 1024], F32, tag="s2", bufs=2,
                                 name=f"s1_{hp}")
                    nc.vector.tensor_add(out=s1[:], in0=t1[0:32, :],
                                         in1=m2[:])
                    rinv1 = pb.tile([32, 1024], F32, tag="s2", bufs=2,
                                    name=f"ri1_{hp}")
                    nc.vector.reciprocal_approx_fast(out=rinv1[:], in_=s1[:])
                    rb1 = pb.tile([128, 1024], BF16, tag="rb1", bufs=2,
                                  name=f"rb1_{hp}")
                    nc.scalar.copy(rb1[0:32, :], rinv1[:])
                    nc.sync.dma_start(out=rb1[32:64, :], in_=rb1[0:32, :])
                    nc.sync.dma_start(out=rb1[64:128, :], in_=rb1[0:64, :])
                    nc.vector.tensor_mul(out=ex1[0:64, :], in0=ex1[0:64, :],
                                         in1=rb1[0:64, :])
                    nc.vector.tensor_mul(out=ex1[64:128, :],
                                         in0=ex1[64:128, :],
                                         in1=rb1[64:128, :])
                    nc.gpsimd.tensor_mul(out=e0[0][:], in0=e0[0][:],
                                         in1=rb0[:])
                    nc.gpsimd.tensor_mul(out=e0[1][:], in0=e0[1][:],
                                         in1=rb0[:])
                    nc.gpsimd.tensor_mul(out=e0[2][:], in0=e0[2][:],
                                         in1=rb0[:])
                    nc.gpsimd.tensor_mul(out=e0[3][:], in0=e0[3][:],
                                         in1=rb0[:])

                    # software-pipeline the o matmuls one head-pair behind so
                    # the tensor queue never waits on the softmax chain.
                    if hp > 0:
                        emit_o(hp - 1)
                emit_o(HP - 1)

            # -------- phase C: out-proj + bias + residual --------
            with tc.tile_pool(name="phc", bufs=1) as pc_pool:
                for fot in range(FT):
                    wo_b = pc_pool.tile([128, FT * 128], BF16, tag="wo",
                                        bufs=2, name=f"wo{fot}")
                    nc.sync.dma_start(
                        out=wo_b.rearrange("p (f o) -> p f o", f=FT),
                        in_=woT_v[:, :, fot * 128:(fot + 1) * 128])
                    xr = pc_pool.tile([128, NCOMP * SL], F32, tag="xr",
                                      bufs=2, name=f"xr{fot}")
                    nc.scalar.dma_start(
                        out=xr.rearrange("p (c s) -> p c s", c=NCOMP),
                        in_=xT_v[:, :, fot, :])
                    ob = pc_pool.tile([128, NCOMP * SL], F32, tag="ob",
                                      bufs=2, name=f"ob{fot}")
                    for c in range(NCOMP):
                        pc = qp.tile([128, 512], F32, tag="pq", bufs=2,
                                     name=f"pc{fot}_{c}")
                        for fi in range(FT):
                            nc.tensor.matmul(
                                pc[:], wo_b[:, fi * 128:(fi + 1) * 128],
                                ot_sb[(c, fi)][:],
                                start=(fi == 0), stop=(fi == FT - 1))
                        eng = nc.vector
                        eng.scalar_tensor_tensor(
                            out=ob[:, c * SL:(c + 1) * SL], in0=pc[:],
                            scalar=bo_sb[:, fot:fot + 1],
                            in1=xr[:, c * SL:(c + 1) * SL],
                            op0=ALU.add, op1=ALU.add)
                    nc.sync.dma_start(
                        out=outT_v[:, :, fot, :],
                        in_=ob.rearrange("p (c s) -> p c s", c=NCOMP))


_NC_CACHE = {}


def _get_nc():
    if "nc" not in _NC_CACHE:
        nc = bacc.Bacc("TRN2", target_bir_lowering=False)
        with tile.TileContext(nc) as tc:
            _emit(tc)
        nc.compile()
        _NC_CACHE["nc"] = nc
    return _NC_CACHE["nc"]


def kernel(hidden_states, encoder_hidden_states, temperature, Wq, Wk, Wv, Wo,
           bo, pad_length):
    # pad branch contributes zero to the output (zeros projected with no
    # bias give k_pad = v_pad = 0), so pad_length is irrelevant.
    hs = np.ascontiguousarray(np.asarray(hidden_states, dtype=np.float32))
    ehs = np.ascontiguousarray(
        np.asarray(encoder_hidden_states, dtype=np.float32))
    temp = float(np.asarray(temperature).reshape(-1)[0])
    Wq = np.asarray(Wq, dtype=np.float32)
    Wk = np.asarray(Wk, dtype=np.float32)
    Wv = np.asarray(Wv, dtype=np.float32)
    Wo = np.asarray(Wo, dtype=np.float32)
    bo_v = np.asarray(bo, dtype=np.float32).reshape(-1)

    wqT = np.ascontiguousarray((Wq / (temp + EPS)).T).astype(ml_dtypes.bfloat16)
    wkT = np.ascontiguousarray(Wk.T).astype(ml_dtypes.bfloat16)
    wvT = np.ascontiguousarray(Wv.T).astype(ml_dtypes.bfloat16)
    woT = np.ascontiguousarray(Wo.T).astype(ml_dtypes.bfloat16)
    eT_all = np.zeros((D, EPCAT), dtype=ml_dtypes.bfloat16)
    for c in range(NCOMP):
        eT_all[:, c * EP:c * EP + E] = ehs[c].T.astype(ml_dtypes.bfloat16)
    bo_t = np.ascontiguousarray(bo_v.reshape(FT, 128).T)

    nc = _get_nc()
    in_maps = []
    for i in range(NCORES):
        xT_i = np.ascontiguousarray(
            hs[:, i * SL:(i + 1) * SL, :].transpose(0, 2, 1))
        in_maps.append({
            "xT": xT_i, "xTb": xT_i.astype(ml_dtypes.bfloat16),
            "eT": eT_all, "wqT": wqT, "wkT": wkT,
            "wvT": wvT, "woT": woT, "bo": bo_t,
        })

    res = run_bass_kernel_spmd(nc, in_maps, core_ids=list(range(NCORES)))

    out = np.empty((NCOMP, S, D), dtype=np.float32)
    for i in range(NCORES):
        out[:, i * SL:(i + 1) * SL, :] = res.results[i]["outT"].transpose(
            0, 2, 1)
    return out


# revision 12
# speedup vs baseline: 1.7297x; 1.0786x over previous
"""Trainium2 Bass kernel for DecomposingAttnProcessor (pad variant).

Math (pad branch contributes exactly zero since pad tokens are zeros
projected with no bias -> k_pad = v_pad = 0):
    q = hs @ Wq.T / (temp + eps)   (scale folded into Wq on host)
    k = ehs @ Wk.T ; v = ehs @ Wv.T
    scores[c,h,s,e] = q . k        (per head, dh=64)
    w = softmax over the 4 components c (dim 0)
    o = w @ v ; out = o @ Wo.T + bo + hs

Sharding: 8 cores, split S=4096 into 512-row blocks; all 4 components of
a block stay on one core (softmax couples them). K/V computed redundantly
per core (encoder seq is only 154).

Device layout is fully transposed (features on partitions). The encoder
axis is zero-padded per component to EP=160 on host so the e-tail
(154-128=26 rows) becomes a clean 32-wide strip: tail scores / V / o
matmuls pack the 4 components into the four 32-col (or 32-row) PE array
tile groups and run concurrently; the pad rows contribute exactly zero
(k_pad = v_pad = 0).
"""

import numpy as np
import ml_dtypes

import concourse.bass as bass
import concourse.mybir as mybir
import concourse.tile as tile
from concourse import bacc
from concourse.bass_utils import run_bass_kernel_spmd

F32 = mybir.dt.float32
BF16 = mybir.dt.bfloat16
AF = mybir.ActivationFunctionType
ALU = mybir.AluOpType

NCOMP = 4
HEADS = 24
DH = 64
D = 1536
S = 4096
E = 154
E0 = 128                  # head chunk of encoder axis
EP = 160                  # per-component padded encoder length
E1 = EP - E0              # 32: tail chunk (rows 154..159 are zero pad)
EPCAT = NCOMP * EP        # 640
EH = EPCAT // 2           # 320: K^T psum chunk
EPS = 1e-8
NCORES = 8
SL = S // NCORES          # 512 s-rows per core (per component)
FT = D // 128             # 12 feature tiles of 128
HP = HEADS // 2           # 12 head-pairs (2 heads = 128 feature rows)


def _emit(tc):
    nc = tc.nc

    xTb = nc.declare_dram_parameter("xTb", [NCOMP, D, SL], BF16, isOutput=False)
    ident = nc.declare_dram_parameter("ident", [128, 128], BF16,
                                      isOutput=False)
    eT = nc.declare_dram_parameter("eT", [D, EPCAT], BF16, isOutput=False)
    wqT = nc.declare_dram_parameter("wqT", [D, D], BF16, isOutput=False)
    wkT = nc.declare_dram_parameter("wkT", [D, D], BF16, isOutput=False)
    wvT = nc.declare_dram_parameter("wvT", [D, D], BF16, isOutput=False)
    woT = nc.declare_dram_parameter("woT", [D, D], BF16, isOutput=False)
    bo = nc.declare_dram_parameter("bo", [128, FT], F32, isOutput=False)
    outT = nc.declare_dram_parameter("outT", [NCOMP, D, SL], F32, isOutput=True)

    # DRAM views with the 128-row tile index folded into the free dim.
    xTb_v = [xTb[c].rearrange("(f p) s -> p f s", p=128) for c in range(NCOMP)]
    xTb_vv = xTb.rearrange("c (f p) s -> p c f s", p=128)
    eT_v = eT.rearrange("(f p) e -> p f e", p=128)
    wqT_v = wqT.rearrange("(f p) o -> p f o", p=128)
    wkT_v = wkT.rearrange("(f p) o -> p f o", p=128)
    wvT_v = wvT.rearrange("(f p) o -> p f o", p=128)
    woT_v = woT.rearrange("(f p) o -> p f o", p=128)
    outT_v = outT.rearrange("c (f p) s -> p c f s", p=128)

    with (
        tc.tile_pool(name="persist", bufs=1) as pp,
        tc.tile_pool(name="psum", bufs=1, space="PSUM") as qp,
    ):
        # ---------------- persistent tiles ----------------
        # K^T per head-pair: partitions = the 128 K-features of the pair,
        # free = c*EP + e.
        kt_sb = [pp.tile([128, EPCAT], BF16, tag="kT", bufs=FT, name=f"kt{t}")
                 for t in range(FT)]
        # V head chunk: partitions = e rows 0..127 of comp c, free = all
        # 1536 v-features.
        v0_sb = [pp.tile([128, D], BF16, tag="v0", bufs=NCOMP, name=f"v0_{c}")
                 for c in range(NCOMP)]
        # V tail: partitions 32c..32c+31 = e rows 128..159 of comp c
        # (rows >=154 are zeros).
        v1_sb = pp.tile([128, D], BF16, tag="v1", bufs=1, name="v1_sb")
        bo_sb = pp.tile([128, FT], F32, tag="bo", bufs=1, name="bo_sb")
        id_sb = pp.tile([128, 128], BF16, tag="ident", bufs=1, name="id_sb")
        nc.scalar.dma_start(out=bo_sb[:], in_=bo[:])
        nc.scalar.dma_start(out=id_sb[:], in_=ident[:])

        def _copy3(i, out, in_):
            # alternate PSUM->SBUF copies over vector/scalar (gpsimd cannot
            # read PSUM)
            if i % 2 == 0:
                nc.vector.tensor_copy(out=out, in_=in_)
            else:
                nc.scalar.copy(out, in_)


        # ---------------- phase A: K^T and V ----------------
        with tc.tile_pool(name="pha", bufs=1) as pa:
            et_b = pa.tile([128, FT * EPCAT], BF16, tag="eT", bufs=1,
                           name="et_b")
            nc.sync.dma_start(
                out=et_b.rearrange("p (f e) -> p f e", f=FT), in_=eT_v)
            et = [et_b[:, fi * EPCAT:(fi + 1) * EPCAT] for fi in range(FT)]

            # K^T[fo, c*EP + e] over fi; N=640 split 320+320 per psum bank.
            for fot in range(FT):
                wk_b = pa.tile([128, FT * 128], BF16, tag="wk", bufs=3,
                               name=f"wk{fot}")
                nc.sync.dma_start(
                    out=wk_b.rearrange("p (f o) -> p f o", f=FT),
                    in_=wkT_v[:, :, fot * 128:(fot + 1) * 128])
                pk = qp.tile([128, 1024], F32, tag="ps2", bufs=2,
                             name=f"pk{fot}")
                for nch in range(2):
                    for fi in range(FT):
                        nc.tensor.matmul(
                            pk[:, nch * 512:nch * 512 + EH],
                            wk_b[:, fi * 128:(fi + 1) * 128],
                            et[fi][:, nch * EH:(nch + 1) * EH],
                            start=(fi == 0), stop=(fi == FT - 1))
                _copy3(fot, kt_sb[fot].rearrange("p (h n) -> p h n", h=2),
                       pk.rearrange("p (h n) -> p h n", h=2)[:, :, 0:EH])

            # V over fi, 512 v-feature columns at a time.
            for fvc in range(3):
                wv_b = pa.tile([128, FT * 512], BF16, tag="wv", bufs=2,
                               name=f"wv{fvc}")
                nc.sync.dma_start(
                    out=wv_b.rearrange("p (f o) -> p f o", f=FT),
                    in_=wvT_v[:, :, fvc * 512:(fvc + 1) * 512])
                for c in range(NCOMP):
                    pv = qp.tile([128, 512], F32, tag="pq", bufs=2,
                                 name=f"pv{fvc}_{c}")
                    for fi in range(FT):
                        nc.tensor.matmul(
                            pv[:], et[fi][:, c * EP:c * EP + E0],
                            wv_b[:, fi * 512:(fi + 1) * 512],
                            start=(fi == 0), stop=(fi == FT - 1))
                    _copy3(c, v0_sb[c][:, fvc * 512:(fvc + 1) * 512], pv[:])
                # tail: 4 components col-packed into one psum bank
                pv1 = qp.tile([128, 512], F32, tag="po", bufs=2,
                              name=f"pv1_{fvc}")
                for fi in range(FT):
                    for c in range(NCOMP):
                        nc.tensor.matmul(
                            pv1[32 * c:32 * c + E1, :],
                            et[fi][:, c * EP + E0:(c + 1) * EP],
                            wv_b[:, fi * 512:(fi + 1) * 512],
                            start=(fi == 0), stop=(fi == FT - 1),
                            skip_group_check=True,
                            tile_position=(0, 32 * c))
                _copy3(fvc, v1_sb[:, fvc * 512:(fvc + 1) * 512], pv1[:])

        # ---------------- phases B (attention) + C (out-proj) ----------------
        # pot holds the per-head o^T results through phase C; pbt holds the
        # phase-B transients and is released before phase C allocates.
        with tc.tile_pool(name="pot", bufs=1) as pot:
            ot_sb = {}

            def emit_o(hp):
                # o^T for head pair hp: psum partitions hh*64..hh*64+64 per
                # head (col groups), accumulation over the two e-chunks.
                # Emission order: all ei0 (full-K, col-paired), then all ei1
                # (K=32 row strips x col groups -> 8 concurrent matmuls).
                pos = []
                for c in range(NCOMP):
                    po = qp.tile([128, 512], F32, tag="po", bufs=2,
                                 name=f"po{hp}_{c}")
                    for hh in range(2):
                        h = hp * 2 + hh
                        nc.tensor.matmul(
                            po[hh * 64:(hh + 1) * 64, :],
                            v0_sb[c][:, h * 64:(h + 1) * 64],
                            ex0_sb[(hp, c)][:, hh * 512:(hh + 1) * 512],
                            start=True, stop=False, skip_group_check=True)
                    pos.append(po)
                for c in range(NCOMP):
                    for hh in range(2):
                        h = hp * 2 + hh
                        nc.tensor.matmul(
                            pos[c][hh * 64:(hh + 1) * 64, :],
                            v1_sb[32 * c:32 * c + E1, h * 64:(h + 1) * 64],
                            ex1_sb[hp][32 * c:32 * c + E1,
                                       hh * 512:(hh + 1) * 512],
                            start=False, stop=True, skip_group_check=True,
                            tile_position=(32 * c, 64 * hh))
                for c in range(NCOMP):
                    ot = pot.tile([128, 512], BF16, tag="oT", bufs=48,
                                  name=f"ot{hp}_{c}")
                    if c % 2 == 0:
                        nc.vector.tensor_copy(out=ot[:], in_=pos[c][:])
                    else:
                        nc.scalar.copy(ot[:], pos[c][:])
                    ot_sb[(c, hp)] = ot

            ex0_sb = {}
            ex1_sb = {}
            with tc.tile_pool(name="pbt", bufs=1) as pb:
                # x panels: on the sync queue AFTER the phase-A loads so the
                # critical-path eT/wk transfers get the bandwidth first.
                xh = []
                for c in range(NCOMP):
                    t = pb.tile([128, FT * SL], BF16, tag="xh", bufs=NCOMP,
                                name=f"xh{c}")
                    nc.sync.dma_start(
                        out=t.rearrange("p (f s) -> p f s", f=FT),
                        in_=xTb_v[c])
                    xh.append(t)

                for hp in range(HP):
                    wq_b = pb.tile([128, FT * 128], BF16, tag="wq", bufs=2,
                                   name=f"wq{hp}")
                    nc.sync.dma_start(
                        out=wq_b.rearrange("p (f o) -> p f o", f=FT),
                        in_=wqT_v[:, :, hp * 128:(hp + 1) * 128])

                    # Q^T for the two heads of this pair, all 4 components
                    qt = []
                    for c in range(NCOMP):
                        pq = qp.tile([128, 512], F32, tag="pq", bufs=2,
                                     name=f"pq{hp}_{c}")
                        for fi in range(FT):
                            nc.tensor.matmul(
                                pq[:], wq_b[:, fi * 128:(fi + 1) * 128],
                                xh[c][:, fi * SL:(fi + 1) * SL],
                                start=(fi == 0), stop=(fi == FT - 1))
                        q = pb.tile([128, SL], BF16, tag="qT", bufs=4,
                                    name=f"qt{hp}_{c}")
                        nc.scalar.copy(q[:], pq[:])
                        qt.append(q)

                    # scores head-chunk: per c one 2-bank psum, the two heads
                    # row-packed (K=64 at row offsets 0/64).
                    for c in range(NCOMP):
                        ps = qp.tile([128, 1024], F32, tag="ps2", bufs=2,
                                     name=f"ps{hp}_{c}")
                        for hh in range(2):
                            nc.tensor.matmul(
                                ps[:, hh * 512:(hh + 1) * 512],
                                kt_sb[hp][hh * 64:(hh + 1) * 64,
                                          c * EP:c * EP + E0],
                                qt[c][hh * 64:(hh + 1) * 64, :],
                                start=True, stop=True)
                        ex = pb.tile([128, 1024], BF16, tag="ex0", bufs=12,
                                     name=f"ex0_{hp}_{c}")
                        for hh in range(2):
                            nc.scalar.activation(
                                ex[:, hh * 512:(hh + 1) * 512],
                                ps[:, hh * 512:(hh + 1) * 512], AF.Exp)
                        ex0_sb[(hp, c)] = ex

                    # scores tail: 8 concurrent matmuls (rows 64*hh x col
                    # group c), all into one 2-bank psum.
                    ps1 = qp.tile([128, 1024], F32, tag="ps2", bufs=2,
                                  name=f"ps1_{hp}")
                    for c in range(NCOMP):
                        for hh in range(2):
                            nc.tensor.matmul(
                                ps1[32 * c:32 * c + E1,
                                    hh * 512:(hh + 1) * 512],
                                kt_sb[hp][hh * 64:(hh + 1) * 64,
                                          c * EP + E0:(c + 1) * EP],
                                qt[c][hh * 64:(hh + 1) * 64, :],
                                start=True, stop=True,
                                skip_group_check=True,
                                tile_position=(64 * hh, 32 * c))
                    ex1 = pb.tile([128, 1024], BF16, tag="ex1", bufs=3,
                                  name=f"ex1_{hp}")
                    for hh in range(2):
                        nc.scalar.activation(
                            ex1[:, hh * 512:(hh + 1) * 512],
                            ps1[:, hh * 512:(hh + 1) * 512], AF.Exp)
                    ex1_sb[hp] = ex1

                    # cross-component softmax: sum the 4 exps, fast
                    # reciprocal, scale each exp (weights stay in-place).
                    e0 = [ex0_sb[(hp, c)] for c in range(NCOMP)]
                    sa = pb.tile([128, 1024], F32, tag="sa", bufs=2,
                                 name=f"sa{hp}")
                    sb_ = pb.tile([128, 1024], F32, tag="sb", bufs=1,
                                  name=f"sb{hp}")
                    nc.vector.tensor_add(out=sa[:], in0=e0[0][:], in1=e0[1][:])
                    nc.gpsimd.tensor_add(out=sb_[:], in0=e0[2][:],
                                         in1=e0[3][:])
                    nc.vector.tensor_add(out=sa[:], in0=sa[:], in1=sb_[:])
                    rinv0 = pb.tile([128, 1024], F32, tag="sa", bufs=2,
                                    name=f"ri0_{hp}")
                    nc.vector.reciprocal_approx_fast(out=rinv0[:], in_=sa[:])
                    rb0 = pb.tile([128, 1024], BF16, tag="rb0", bufs=2,
                                  name=f"rb0_{hp}")
                    nc.scalar.copy(rb0[:], rinv0[:])

                    # tail: components live on 32-partition strips (e-row
                    # j of comp c at partition 32c+j). Element-wise engines
                    # need same-partition operands, so the cross-strip moves
                    # go through SBUF->SBUF DMA.
                    m1 = pb.tile([64, 1024], BF16, tag="m1", bufs=2,
                                 name=f"m1_{hp}")
                    nc.sync.dma_start(out=m1[:], in_=ex1[64:128, :])
                    t1 = pb.tile([64, 1024], BF16, tag="s1", bufs=2,
                                 name=f"t1_{hp}")
                    nc.vector.tensor_add(out=t1[:], in0=ex1[0:64, :],
                                         in1=m1[:])
                    m2 = pb.tile([32, 1024], BF16, tag="m2", bufs=2,
                                 name=f"m2_{hp}")
                    nc.sync.dma_start(out=m2[:], in_=t1[32:64, :])
                    s1 = pb.tile([32, 1024], F32, tag="s2", bufs=2,
                                 name=f"s1_{hp}")
                    nc.vector.tensor_add(out=s1[:], in0=t1[0:32, :],
                                         in1=m2[:])
                    rinv1 = pb.tile([32, 1024], F32, tag="s2", bufs=2,
                                    name=f"ri1_{hp}")
                    nc.vector.reciprocal_approx_fast(out=rinv1[:], in_=s1[:])
                    rb1 = pb.tile([128, 1024], BF16, tag="rb1", bufs=2,
                                  name=f"rb1_{hp}")
                    nc.scalar.copy(rb1[0:32, :], rinv1[:])
                    nc.sync.dma_start(out=rb1[32:64, :], in_=rb1[0:32, :])
                    nc.sync.dma_start(out=rb1[64:128, :], in_=rb1[0:64, :])
                    nc.vector.tensor_mul(out=ex1[0:64, :], in0=ex1[0:64, :],
                                         in1=rb1[0:64, :])
                    nc.vector.tensor_mul(out=ex1[64:128, :],
                                         in0=ex1[64:128, :],
                                         in1=rb1[64:128, :])
                    nc.gpsimd.tensor_mul(out=e0[0][:], in0=e0[0][:],
                                         in1=rb0[:])
                    nc.gpsimd.tensor_mul(out=e0[1][:], in0=e0[1][:],
                                         in1=rb0[:])
                    nc.gpsimd.tensor_mul(out=e0[2][:], in0=e0[2][:],
                                         in1=rb0[:])
                    nc.gpsimd.tensor_mul(out=e0[3][:], in0=e0[3][:],
                                         in1=rb0[:])

                    # software-pipeline the o matmuls two head-pairs behind
                    # so the tensor queue never waits on the softmax chain
                    # (which includes several SBUF->SBUF DMA hops).
                    if hp >= 2:
                        emit_o(hp - 2)
                emit_o(HP - 2)
                emit_o(HP - 1)

            # -------- phase C: out-proj + bias + residual --------
            # residual is accumulated into PSUM via an identity matmul on the
            # bf16 x panel; bias comes in through the scalar-engine
            # activation, which also does the PSUM->SBUF copy.
            with tc.tile_pool(name="phc", bufs=1) as pc_pool:
                for fot in range(FT):
                    wo_b = pc_pool.tile([128, FT * 128], BF16, tag="wo",
                                        bufs=3, name=f"wo{fot}")
                    nc.sync.dma_start(
                        out=wo_b.rearrange("p (f o) -> p f o", f=FT),
                        in_=woT_v[:, :, fot * 128:(fot + 1) * 128])
                    xrb = pc_pool.tile([128, NCOMP * SL], BF16, tag="xrb",
                                       bufs=2, name=f"xrb{fot}")
                    nc.scalar.dma_start(
                        out=xrb.rearrange("p (c s) -> p c s", c=NCOMP),
                        in_=xTb_vv[:, :, fot, :])
                    ob = pc_pool.tile([128, NCOMP * SL], F32, tag="ob",
                                      bufs=2, name=f"ob{fot}")
                    for c in range(NCOMP):
                        pc = qp.tile([128, 512], F32, tag="pq", bufs=2,
                                     name=f"pc{fot}_{c}")
                        for fi in range(FT):
                            nc.tensor.matmul(
                                pc[:], wo_b[:, fi * 128:(fi + 1) * 128],
                                ot_sb[(c, fi)][:],
                                start=(fi == 0), stop=False)
                        nc.tensor.matmul(
                            pc[:], id_sb[:],
                            xrb[:, c * SL:(c + 1) * SL],
                            start=False, stop=True)
                        nc.scalar.activation(
                            ob[:, c * SL:(c + 1) * SL], pc[:], AF.Identity,
                            bias=bo_sb[:, fot:fot + 1])
                    nc.scalar.dma_start(
                        out=outT_v[:, :, fot, :],
                        in_=ob.rearrange("p (c s) -> p c s", c=NCOMP))


_NC_CACHE = {}


def _get_nc():
    if "nc" not in _NC_CACHE:
        nc = bacc.Bacc("TRN2", target_bir_lowering=False)
        with tile.TileContext(nc) as tc:
            _emit(tc)
        nc.compile()
        _NC_CACHE["nc"] = nc
    return _NC_CACHE["nc"]


def kernel(hidden_states, encoder_hidden_states, temperature, Wq, Wk, Wv, Wo,
           bo, pad_length):
    # pad branch contributes zero to the output (zeros projected with no
    # bias give k_pad = v_pad = 0), so pad_length is irrelevant.
    hs = np.ascontiguousarray(np.asarray(hidden_states, dtype=np.float32))
    ehs = np.ascontiguousarray(
        np.asarray(encoder_hidden_states, dtype=np.float32))
    temp = float(np.asarray(temperature).reshape(-1)[0])
    Wq = np.asarray(Wq, dtype=np.float32)
    Wk = np.asarray(Wk, dtype=np.float32)
    Wv = np.asarray(Wv, dtype=np.float32)
    Wo = np.asarray(Wo, dtype=np.float32)
    bo_v = np.asarray(bo, dtype=np.float32).reshape(-1)

    wqT = np.ascontiguousarray((Wq / (temp + EPS)).T).astype(ml_dtypes.bfloat16)
    wkT = np.ascontiguousarray(Wk.T).astype(ml_dtypes.bfloat16)
    wvT = np.ascontiguousarray(Wv.T).astype(ml_dtypes.bfloat16)
    woT = np.ascontiguousarray(Wo.T).astype(ml_dtypes.bfloat16)
    eT_all = np.zeros((D, EPCAT), dtype=ml_dtypes.bfloat16)
    for c in range(NCOMP):
        eT_all[:, c * EP:c * EP + E] = ehs[c].T.astype(ml_dtypes.bfloat16)
    bo_t = np.ascontiguousarray(bo_v.reshape(FT, 128).T)

    ident_np = np.eye(128, dtype=ml_dtypes.bfloat16)

    nc = _get_nc()
    in_maps = []
    for i in range(NCORES):
        xT_i = np.ascontiguousarray(
            hs[:, i * SL:(i + 1) * SL, :].transpose(0, 2, 1))
        in_maps.append({
            "xTb": xT_i.astype(ml_dtypes.bfloat16),
            "eT": eT_all, "wqT": wqT, "wkT": wkT,
            "wvT": wvT, "woT": woT, "bo": bo_t, "ident": ident_np,
        })

    res = run_bass_kernel_spmd(nc, in_maps, core_ids=list(range(NCORES)))

    out = np.empty((NCOMP, S, D), dtype=np.float32)
    for i in range(NCORES):
        out[:, i * SL:(i + 1) * SL, :] = res.results[i]["outT"].transpose(
            0, 2, 1)
    return out
